# revision 29
# baseline (speedup 1.0000x reference)
"""NT-Xent loss on 8 TRN2 NeuronCores — symmetric fp8 version.

Reference computes, for z = concat(z1, z2) (2N=8192 rows, D=256):
    zn  = z / max(||z||, eps)
    sim = (zn @ zn.T) / T, diag masked to -1e9
    loss = mean_i( logsumexp_j sim[i, j] - sim[i, pos(i)] ),  pos(i) = (i + N) % 2N

Strategy vs the plain row-sharded version:
  * sim is SYMMETRIC, so each core only computes column-strips at block
    distance 0..8 of its own rows (4608 of 8192 columns per row); the
    transposed contributions (distance 9..15) are recovered as COLUMN
    sums of the exp'd tiles from the cores that did compute them.
    This cuts the ScalarE exp work (the kernel bottleneck) by ~45%.
  * Matmuls run in fp8e4 DoubleRow perf mode: zn is scaled by 16,
    quantized to fp8, and laid out in [128, 2, cols] contract-pairs, so
    the full 256-deep contraction is a single PE pass at 2 MACs/cell.
    This also halves the HBM input traffic (the DMA head).
  * The self-similarity diagonal is killed inside PSUM by one extra
    accumulating matmul (16*I)^T @ (-240*I) = -3840*I, which drives
    exp() to 0 exactly — no host-side cancellation needed.
  * The exp'd tiles are written as bf16; column sums accumulate across
    the whole kernel into a single [16, 512] PSUM bank via tiny
    selector-weight matmuls.
  * Positive-pair logits are the diagonal of the distance-8 strip,
    extracted with an eye-mask multiply + reduce on VectorE.
  * Row sums ride on the ACTIVATE accumulator (accum_out), materialized
    into PSUM (ScalarE's faster port: READ_ACCUMULATOR 284 -> 226 ns).

Layout per core (SPMD; data rotated by c*1024 columns on the host so all
cores run the identical program):
  rows: 1024 (8 m-tiles of 128). 16 global col-strips of 512.
  m-tiles 0-3 (row-strip 0) read col-strips 0..8  (abs cols [0, 4608)),
  m-tiles 4-7 (row-strip 1) read col-strips 1..9  (abs cols [512, 5120)).
  Per m: 3 PSUM tiles [128, 1536] -> 3 exp+rowsum ACTIVATEs (m=0 is
  split 512/1024/1536/1536 so ScalarE starts as soon as the first DMA
  chunks land).
  Col-strip k of core c holds global rows ((2c+k) % 16) * 512 + j.
Host combines rowsums + colsums + pexp into the scalar loss in float64.

Measured on TRN2 (8 cores, core-0 NTFF span): ~61-62 us vs 92.4 us for
the full-matrix bf16 version; rel err vs the fp64 reference ~9e-5.
Steady state is ScalarE-bound (exp at 1 elem/cycle/lane + ~260 cyc per
ACTIVATE overhead); PE runs ~0.4 us/m-tile behind it. Remaining fixed
costs: ~7 us framework preamble, ~3.5 us DMA head (HBM-bound: 8 cores x
1.37 MB replicated input), ~1.8 us tail, ~9.7 us multi-core teardown.

Scheduling notes (hard-won, from NTFF traces):
  * Colsum matmuls are deferred one m-iteration: the PE queue is FIFO,
    so colsums waiting on this m's ACTIVATEs would stall the next m's
    data matmuls and bubble the act stream (~300-470 ns per m).
  * q1/q2 rowsums of m<7 use VectorE reduce_sum instead of the ACT
    accumulator (READ_ACCUMULATOR costs a ~160-275 ns ScalarE slot);
    m=7 keeps ACT accum because a trailing 1.8 us DVE reduce would
    gate the output DMAs.
  * m=0 is split 512/1024/1536/1536 and m=7 1536/1536/1024/512 so the
    first ACTIVATEs wait only on the first DMA chunks and the last
    (short) ACTIVATE covers only the d8 strip, letting the colsum
    copy/DMA overlap it.
  * znt chunks ship as separate contiguous DRAM tensors (a sliced
    [128, 2, 5120] AP produced 256 strided 512 B DMA segments, far
    below line rate) and ride the sync HWDGE queue; the gpsimd SWDGE
    queue is slower and only carries late-needed data.
"""

import sys

if "/opt/trn_rl_repo" not in sys.path:
    sys.path.insert(0, "/opt/trn_rl_repo")

import ml_dtypes
import numpy as np

import concourse.bass as bass
import concourse.mybir as mybir
import concourse.tile as tile
from concourse import bacc
from concourse.bass_utils import run_bass_kernel_spmd

N = 4096
D = 256
TWO_N = 2 * N
TEMPERATURE = 0.07
EPS = 1e-8
N_CORES = 8
ROWS_PER_CORE = TWO_N // N_CORES   # 1024
M_TILES = ROWS_PER_CORE // 128     # 8
FP8_SCALE = 16.0                   # zn pre-scale before fp8 quantization
ACT_SCALE = 1.0 / (TEMPERATURE * FP8_SCALE * FP8_SCALE)
STRIP = 512                        # column strip width
WINDOW = 9 * STRIP                 # 4608 columns of exp work per row
SPAN = 10 * STRIP                  # 5120 columns of znt needed per core
QT = 1536                          # act/psum tile width (3 per m-tile)


def _act_layout():
    """Per-m act-tile spans plus rowsum routing (ACT accum vs VectorE).

    m=0 is split finer so the first ACTIVATEs only wait on the first DMA
    chunks. Later tiles of m<7 reduce their rowsums on the mostly-idle
    VectorE (saves the READ_ACCUMULATOR slot on the saturated ScalarE);
    m=7 stays fully on the ACT accumulator so the slow DVE reduce never
    gates the kernel tail.
    Returns (spans[m], route[m]) where route[m][ti] = ("acc"|"dve", idx).
    """
    spans_all, route_all = [], []
    ai = di = 0
    for m in range(M_TILES):
        woff = STRIP * (m // 4)
        if m == 0:
            spans = [(0, 512), (512, 1536), (1536, 3072), (3072, 4608)]
        elif m == M_TILES - 1:
            # last m-tile: keep the d8/pexp strip as its own final
            # (short) act so the d6/d7 colsums + cs copy/DMA finish
            # during it instead of trailing the kernel.
            spans = [(512, 2048), (2048, 3584), (3584, 4608), (4608, 5120)]
        else:
            spans = [(woff + QT * q, woff + QT * (q + 1)) for q in range(3)]
        route = []
        for ti in range(len(spans)):
            if m < 7 and ti >= (2 if m == 0 else 1):
                route.append(("dve", di))
                di += 1
            else:
                route.append(("acc", ai))
                ai += 1
        spans_all.append(spans)
        route_all.append(route)
    return spans_all, route_all, ai, di


SPANS_ALL, ROUTE_ALL, N_ACC, N_DVE = _act_layout()

_cached = {}


def _build_bass():
    f32 = mybir.dt.float32
    bf16 = mybir.dt.bfloat16
    fp8 = mybir.dt.float8e4
    DR = mybir.MatmulPerfMode.DoubleRow
    nc = bacc.Bacc("TRN2", target_bir_lowering=False, debug=False)

    # inputs: znt is pre-split on the host into per-chunk contiguous
    # tensors so each DMA moves 128 lines of >=1KB contiguous bytes
    # (slicing one big [128, 2, SPAN] tensor gave 256 strided 512B
    # segments per chunk, well below DMA line-rate).
    BOUNDS = [0, 512, 1536, 3072, 4608, SPAN]
    znt_chunks = [
        nc.declare_dram_parameter(
            f"znt{ci}", [128, 2, BOUNDS[ci + 1] - BOUNDS[ci]], fp8, isOutput=False
        )
        for ci in range(len(BOUNDS) - 1)
    ]
    eyepair = nc.declare_dram_parameter("eyepair", [128, 256], fp8, isOutput=False)
    eye = nc.declare_dram_parameter("eye", [128, 128], bf16, isOutput=False)
    sel = nc.declare_dram_parameter("sel", [128, 128], bf16, isOutput=False)
    # outputs
    acc_out = nc.declare_dram_parameter("acc", [128, N_ACC], f32, isOutput=True)
    dve_out = nc.declare_dram_parameter("dveacc", [128, N_DVE], f32, isOutput=True)
    pexp_out = nc.declare_dram_parameter("pexp", [128, M_TILES], f32, isOutput=True)
    cs_out = nc.declare_dram_parameter("cs", [16, STRIP], f32, isOutput=True)

    with tile.TileContext(nc) as tc:
        with (
            tc.tile_pool(name="sb", bufs=1) as sb,
            tc.tile_pool(name="ps", bufs=1, space=bass.MemorySpace.PSUM) as pp,
        ):
            # znt chunks sized so the m=0 pipeline starts as early as
            # possible: q0 needs [0,1536), q1 [1536,3072), q2 [3072,4608).
            # Interleave across the two DMA queues in consumption order;
            # eyepair (needed by the q0 diag-kill) goes first on gpsimd.
            eyepair_t = sb.tile([128, 256], fp8, tag="eyepair")
            nc.gpsimd.dma_start(eyepair_t[:], eyepair[:])

            zt = [None] * (len(BOUNDS) - 1)
            # chunks 0-3 all go on the sync HWDGE queue (the gpsimd SWDGE
            # path is noticeably slower); the act stream consumes slower
            # than sync delivers. gpsimd only carries late-needed data.
            for ci, eng in ((0, nc.sync), (1, nc.sync), (2, nc.sync),
                            (3, nc.sync), (4, nc.gpsimd)):
                c0, c1 = BOUNDS[ci], BOUNDS[ci + 1]
                t = sb.tile([128, 2, c1 - c0], fp8, tag=f"z{ci}")
                eng.dma_start(t[:, :, :], znt_chunks[ci][:, :, :])
                zt[ci] = t

            eye_t = sb.tile([128, 128], bf16, tag="eye")
            nc.sync.dma_start(eye_t[:], eye[:])
            sel_t = sb.tile([128, 128], bf16, tag="sel")
            nc.gpsimd.dma_start(sel_t[:], sel[:])

            acc_ps = pp.tile([128, N_ACC], f32, tag="accps")
            acc_t = sb.tile([128, N_ACC], f32, tag="acc")
            dve_t = sb.tile([128, N_DVE], f32, tag="dveacc")
            pexp_t = sb.tile([128, M_TILES], f32, tag="pexp")
            cs_ps = pp.tile([16, STRIP], f32, tag="cs")
            cs_sb = sb.tile([16, STRIP], f32, tag="cs_sb")

            def chunk_slice(abs_col, width):
                for ci in range(len(BOUNDS) - 1):
                    if BOUNDS[ci] <= abs_col and abs_col + width <= BOUNDS[ci + 1]:
                        rel = abs_col - BOUNDS[ci]
                        return zt[ci][:, :, rel:rel + width]
                raise AssertionError(f"slice [{abs_col}, {abs_col + width}) crosses chunks")

            for m in range(M_TILES):
                s = m // 4
                woff = STRIP * s
                spans = SPANS_ALL[m]

                def span_slice(abs_col, width):
                    for ti, (a, b) in enumerate(spans):
                        if a <= abs_col and abs_col + width <= b:
                            return ti, abs_col - a
                    raise AssertionError(f"[{abs_col},{abs_col + width}) not in spans")

                exs = []
                dk_tile, dk_rel = span_slice(128 * m, 128)
                if m == 0:
                    pending_cs = []
                for ti, (a, b) in enumerate(spans):
                    ps = pp.tile([128, QT], f32, tag="ps", bufs=2)
                    w = b - a
                    for j in range(w // 512):
                        abs_col = a + 512 * j
                        nc.tensor.matmul(
                            ps[:, 512 * j:512 * j + 512],
                            lhsT=chunk_slice(128 * m, 128),
                            rhs=chunk_slice(abs_col, 512),
                            start=True,
                            stop=not (ti == dk_tile and j == dk_rel // 512),
                            perf_mode=DR,
                        )
                    if ti == dk_tile:
                        # kill the self-similarity diagonal: += -3840*I
                        nc.tensor.matmul(
                            ps[:, dk_rel:dk_rel + 128],
                            lhsT=eyepair_t[:, 0:128],
                            rhs=eyepair_t[:, 128:256],
                            start=False,
                            stop=True,
                        )
                    ex = sb.tile([128, QT], bf16, tag="ex", bufs=8)
                    kind, idx = ROUTE_ALL[m][ti]
                    nc.scalar.activation(
                        out=ex[:, 0:w],
                        in_=ps[:, 0:w],
                        func=mybir.ActivationFunctionType.Exp,
                        bias=0.0,
                        scale=ACT_SCALE,
                        accum_out=None if kind == "dve"
                        else acc_ps[:, idx:idx + 1],
                    )
                    if kind == "dve":
                        nc.vector.reduce_sum(
                            dve_t[:, idx:idx + 1], ex[:, 0:w],
                            axis=mybir.AxisListType.X,
                        )
                    exs.append(ex)

                # positive-pair logits: diagonal of the distance-8 strip
                # (abs cols [woff+4096+128*(m%4), +128)), in the last tile.
                pti, prel = span_slice(woff + 4096 + 128 * (m % 4), 128)
                sc = sb.tile([128, 128], f32, tag="sc", bufs=2)
                nc.vector.tensor_tensor(
                    sc[:], exs[pti][:, prel:prel + 128], eye_t[:],
                    mybir.AluOpType.mult,
                )
                nc.vector.reduce_sum(
                    pexp_t[:, m:m + 1], sc[:], axis=mybir.AxisListType.X
                )

                # column sums of distance 1..7 strips into cs_ps row
                # k = local col-strip index. Deferred by one m-iteration:
                # the PE queue is FIFO, so emitting them here (they wait
                # on this m's ACTIVATEs) would stall the next m's data
                # matmuls and open a bubble in the act stream. One
                # iteration later their ex tiles are guaranteed ready.
                cs_specs = []
                for d in range(1, 8):
                    k = d + s
                    ti, rel = span_slice(woff + 512 * d, 512)
                    cs_specs.append((sel_t[:, 16 * (k - 1):16 * k],
                                     exs[ti][:, rel:rel + 512]))
                for i, (lw, rhs) in enumerate(pending_cs):
                    nc.tensor.matmul(
                        cs_ps[:, :], lhsT=lw, rhs=rhs,
                        start=(m == 1 and i == 0), stop=False,
                    )
                pending_cs = cs_specs

            # flush the deferred colsums of the last m-tile
            for i, (lw, rhs) in enumerate(pending_cs):
                nc.tensor.matmul(
                    cs_ps[:, :], lhsT=lw, rhs=rhs,
                    start=False, stop=(i == len(pending_cs) - 1),
                )

            # DMA cannot read PSUM; stage through SBUF. VectorE is busy
            # with the m=7 pexp extraction, so both copies go to ScalarE
            # (idle after the last READ_ACCUMULATOR); DMAs split queues.
            nc.vector.tensor_copy(cs_sb[:], cs_ps[:])
            nc.scalar.copy(acc_t[:], acc_ps[:])
            nc.sync.dma_start(acc_out[:], acc_t[:])
            nc.sync.dma_start(pexp_out[:], pexp_t[:])
            nc.sync.dma_start(dve_out[:], dve_t[:])
            nc.gpsimd.dma_start(cs_out[:], cs_sb[:])

    nc.compile()
    return nc


def _prepare_inputs(z1, z2):
    z = np.concatenate([np.asarray(z1), np.asarray(z2)], axis=0).astype(np.float32)
    norms = np.maximum(np.sqrt((z.astype(np.float64) ** 2).sum(-1)), EPS)
    zn = (z / norms[:, None]).astype(np.float32)
    q = np.clip(zn * FP8_SCALE, -240.0, 240.0).astype(ml_dtypes.float8_e4m3)
    # paired layout: znt_p[p, i, j] = q[j, 128*i + p]
    znt_p = np.ascontiguousarray(q.T.reshape(2, 128, TWO_N).transpose(1, 0, 2))

    ey = np.eye(128, dtype=np.float32)
    eyepair = np.concatenate(
        [(16.0 * ey), (-240.0 * ey)], axis=1
    ).astype(ml_dtypes.float8_e4m3)
    ey = ey.astype(ml_dtypes.bfloat16)
    sel = np.zeros((128, 128), dtype=ml_dtypes.bfloat16)
    for k in range(1, 9):
        sel[:, 16 * (k - 1) + k] = 1.0

    BOUNDS = [0, 512, 1536, 3072, 4608, SPAN]
    in_maps = []
    for c in range(N_CORES):
        znt_c = np.roll(znt_p, -c * ROWS_PER_CORE, axis=2)[:, :, :SPAN]
        m = {"eyepair": eyepair, "eye": ey, "sel": sel}
        for ci in range(len(BOUNDS) - 1):
            m[f"znt{ci}"] = np.ascontiguousarray(
                znt_c[:, :, BOUNDS[ci]:BOUNDS[ci + 1]]
            )
        in_maps.append(m)
    return in_maps


def kernel(z1, z2):
    if "nc" not in _cached:
        _cached["nc"] = _build_bass()
    nc = _cached["nc"]
    in_maps = _prepare_inputs(z1, z2)
    res = run_bass_kernel_spmd(nc, in_maps, core_ids=list(range(N_CORES)))
    results = res.results

    denom = np.zeros(TWO_N, dtype=np.float64)
    pexp = np.zeros(TWO_N, dtype=np.float64)
    for c in range(N_CORES):
        acc = np.asarray(results[c]["acc"], dtype=np.float64)    # [128, N_ACC]
        dve = np.asarray(results[c]["dveacc"], dtype=np.float64)  # [128, N_DVE]
        px = np.asarray(results[c]["pexp"], dtype=np.float64)    # [128, 8]
        cs = np.asarray(results[c]["cs"], dtype=np.float64)      # [16, 512]
        rows = slice(c * ROWS_PER_CORE, (c + 1) * ROWS_PER_CORE)
        per_m = np.stack(
            [sum(acc[:, i] for k, i in ROUTE_ALL[m] if k == "acc")
             + sum(dve[:, i] for k, i in ROUTE_ALL[m] if k == "dve")
             for m in range(M_TILES)],
            axis=1,
        )  # [128, 8]
        denom[rows] += per_m.T.reshape(-1)
        pexp[rows] = px.T.reshape(-1)
        for k in range(1, 9):
            g0 = ((2 * c + k) % 16) * STRIP
            denom[g0:g0 + STRIP] += cs[k]
    loss_rows = np.log(denom) - np.log(pexp)
    return np.float32(loss_rows.mean())
